# revision 9
# baseline (speedup 1.0000x reference)
"""PointNet++ feature-propagation kernel for 8 Trainium2 NeuronCores.

Algorithm (per batch b): for each of N1=16384 query points, find the 3
nearest of N2=4096 source points (euclidean on xyz), inverse-distance
interpolate their 256-dim features, concat with the query's own 64
features, run a 3-layer MLP (320->256->128->128, relu), emit
[xyz(3), h(128)] = 131 channels.

Sharding: B=4 x (N1 split in 2) = 8 shards, one per core, no
cross-core communication.  Each core processes 8192 query points
against its batch's full 4096 candidates.

Per 128-query tile on device:
  - PE computes S = -d^2 as a K=5 matmul with augmented coords
    (a_ext=[x,y,z,|q|^2,1], b_ext=[2x,2y,2z,-1,-|c|^2]); 4 query
    tiles are packed into distinct 32-row PE strips (tile_position)
    so the fp32 matmuls run concurrently.
  - DVE max/max_index give exact fp32 top-8 values+indices (top-3 used)
  - SWDGE indirect DMA gathers the 3 source-feature rows per query
  - inverse-distance weights + weighted sum, PE transposes to
    channels-on-partitions, fp32 MLP on PE, transpose back, DMA out.
"""

import os
import sys

import numpy as np

for _p in ("/opt/trn_rl_repo", "/root/.axon_site/_ro/trn_rl_repo"):
    if os.path.isdir(_p) and _p not in sys.path:
        sys.path.insert(0, _p)

import concourse.bacc as bacc
import concourse.bass as bass
import concourse.mybir as mybir
import concourse.tile as tile
from concourse.bass import IndirectOffsetOnAxis
from concourse.bass_utils import run_bass_kernel_spmd
from concourse.masks import make_identity

ts = bass.ts
FP = mybir.dt.float32
FPR = mybir.dt.float32r
AF = mybir.ActivationFunctionType
AX = mybir.AxisListType
OP = mybir.AluOpType

B, N1, N2 = 4, 16384, 4096
C1, C2 = 64, 256
CIN, H1, H2, H3 = 320, 256, 128, 128
NCORES = 8
N1S = B * N1 // NCORES  # 8192 queries per core
EPS = 1e-8


def build_program(
    n1s: int = N1S,
    pack4: bool = False,
    stt_gpsimd: bool = False,
    batched_gather: bool = False,
    f32r_mlp: bool = False,
):
    nc = bacc.Bacc(
        "TRN2", target_bir_lowering=False, debug=False, enable_asserts=False
    )
    p1 = nc.dram_tensor("p1", [n1s, 3 + C1], FP, kind="ExternalInput").ap()
    p2xyzt = nc.dram_tensor("p2xyzt", [3, N2], FP, kind="ExternalInput").ap()
    p2f = nc.dram_tensor("p2f", [N2, C2], FP, kind="ExternalInput").ap()
    w1 = nc.dram_tensor("w1", [CIN, H1], FP, kind="ExternalInput").ap()
    w2 = nc.dram_tensor("w2", [H1, H2], FP, kind="ExternalInput").ap()
    w3 = nc.dram_tensor("w3", [H2, H3], FP, kind="ExternalInput").ap()
    b1 = nc.dram_tensor("b1", [H1, 1], FP, kind="ExternalInput").ap()
    b2 = nc.dram_tensor("b2", [H2, 1], FP, kind="ExternalInput").ap()
    b3 = nc.dram_tensor("b3", [H3, 1], FP, kind="ExternalInput").ap()
    out = nc.dram_tensor("out", [n1s, 3 + H3], FP, kind="ExternalOutput").ap()

    ntiles = n1s // 128
    nchunks = N2 // 512
    group = 4 if pack4 else 1
    assert ntiles % group == 0

    def mmdt(ap):
        return ap.bitcast(FPR) if f32r_mlp else ap

    with tile.TileContext(nc, trace_sim=False) as tc:
        with tc.tile_pool(name="consts", bufs=1) as consts:
            ident = consts.tile([128, 128], FP)
            make_identity(nc, ident[:])

            w1s0 = consts.tile([128, H1], FP)
            nc.sync.dma_start(w1s0[:], w1[0:128, :])
            w1s1 = consts.tile([128, H1], FP)
            nc.sync.dma_start(w1s1[:], w1[128:256, :])
            w1s2 = consts.tile([64, H1], FP)
            nc.sync.dma_start(w1s2[:], w1[256:320, :])
            w2s0 = consts.tile([128, H2], FP)
            nc.sync.dma_start(w2s0[:], w2[0:128, :])
            w2s1 = consts.tile([128, H2], FP)
            nc.sync.dma_start(w2s1[:], w2[128:256, :])
            w3s = consts.tile([128, H3], FP)
            nc.sync.dma_start(w3s[:], w3[:, :])
            b1a = consts.tile([128, 1], FP)
            nc.sync.dma_start(b1a[:], b1[0:128, :])
            b1b = consts.tile([128, 1], FP)
            nc.sync.dma_start(b1b[:], b1[128:256, :])
            b2s = consts.tile([128, 1], FP)
            nc.sync.dma_start(b2s[:], b2[:, :])
            b3s = consts.tile([128, 1], FP)
            nc.sync.dma_start(b3s[:], b3[:, :])

            # b_ext = [2x; 2y; 2z; -1; -|c|^2]  ([5, N2]); when packing,
            # replicated into PE row strips {0,32,64,96}.
            p2x = consts.tile([3, N2], FP)
            nc.sync.dma_start(p2x[:], p2xyzt[:, :])
            bext = consts.tile([128 if pack4 else 5, N2], FP)
            sq2 = consts.tile([3, N2], FP)
            nc.vector.tensor_mul(sq2[:], p2x[:], p2x[:])
            ones3 = consts.tile([3, 1], FP)
            nc.gpsimd.memset(ones3[:], 1.0)
            c2row = consts.tile([1, N2], FP)
            with tc.tile_pool(name="setup_psum", bufs=2, space="PSUM") as spp:
                for c in range(nchunks):
                    c2p = spp.tile([1, 512], FP, tag="c2p")
                    nc.tensor.matmul(
                        out=c2p[:], lhsT=ones3[:], rhs=sq2[:, ts(c, 512)],
                        start=True, stop=True,
                    )
                    nc.scalar.mul(c2row[:, ts(c, 512)], c2p[:], -1.0)
            # row 3 must be -1; engines can't address a start-partition of 3,
            # so fill with -1 first and overwrite rows 0-2 and 4.
            nc.vector.memset(bext[0:5, :], -1.0)
            nc.scalar.mul(bext[0:3, :], p2x[:], 2.0)
            # partition 0 -> partition 4 needs a DMA (engines can't shift)
            nc.sync.dma_start(bext[4:5, :], c2row[:, :])
            if pack4:
                for q in range(1, 4):
                    nc.sync.dma_start(bext[32 * q : 32 * q + 5, :], bext[0:5, :])

            with (
                tc.tile_pool(name="p1pool", bufs=2 + 2 * group) as p1pool,
                tc.tile_pool(name="extpool", bufs=2 * group) as extpool,
                tc.tile_pool(name="apool", bufs=2 * group) as apool,
                tc.tile_pool(name="spool", bufs=group + 1) as spool,
                tc.tile_pool(name="small", bufs=2 * group) as small,
                tc.tile_pool(name="gpool", bufs=3) as gpool,
                tc.tile_pool(name="ipool", bufs=3) as ipool,
                tc.tile_pool(name="mpool", bufs=3) as mpool,
                tc.tile_pool(name="opool", bufs=3) as opool,
                tc.tile_pool(name="spsum", bufs=5, space="PSUM") as spsum,
                tc.tile_pool(name="smpsum", bufs=3, space="PSUM") as smpsum,
            ):
                for t0 in range(0, ntiles, group):
                    p1ts, f1Ts, Ssbs = [], [], []
                    if pack4:
                        aext4 = apool.tile([128, 128], FP, tag="aext4")
                    for q in range(group):
                        t = t0 + q
                        p1t = p1pool.tile([128, 3 + C1], FP, tag="p1t")
                        nc.sync.dma_start(p1t[:], p1[ts(t, 128), :])
                        p1ts.append(p1t)

                        # a_ext = [x, y, z, |q|^2, 1] per query
                        ext5 = extpool.tile([128, 5], FP, tag="ext5")
                        sqd = extpool.tile([128, 3], FP, tag="sqd")
                        nc.scalar.activation(
                            sqd[:], p1t[:, 0:3], AF.Square,
                            accum_out=ext5[:, 3:4],
                        )
                        nc.scalar.copy(ext5[:, 0:3], p1t[:, 0:3])
                        nc.gpsimd.memset(ext5[:, 4:5], 1.0)

                        extp5 = smpsum.tile([5, 128], FP, tag="sm")
                        nc.tensor.transpose(extp5[:], ext5[:], ident[:])
                        aext = apool.tile([5, 128], FP, tag="aext")
                        nc.scalar.copy(aext[:], extp5[:])
                        if pack4:
                            # shift into the packed PE row strip (DMA can
                            # move across partitions; engines can't)
                            nc.sync.dma_start(
                                aext4[32 * q : 32 * q + 5, :], aext[:]
                            )

                        f1tp = smpsum.tile([64, 128], FP, tag="sm")
                        nc.tensor.transpose(
                            f1tp[:], p1t[:, 3 : 3 + C1], ident[:]
                        )
                        f1T = apool.tile([64, 128], FP, tag="f1T")
                        nc.scalar.copy(f1T[:], f1tp[:])
                        f1Ts.append(f1T)

                        Ssb = spool.tile([128, N2], FP, tag="Ssb")
                        Ssbs.append(Ssb)



                    # S = -d^2  [128, N2] per query tile
                    for c in range(nchunks):
                        for q in range(group):
                            sps = spsum.tile([128, 512], FP, tag="schunk")
                            if pack4:
                                nc.tensor.matmul(
                                    out=sps[:],
                                    lhsT=aext4[32 * q : 32 * q + 5, :],
                                    rhs=bext[32 * q : 32 * q + 5, ts(c, 512)],
                                    start=True, stop=True,
                                    tile_position=(32 * q, 0),
                                )
                            else:
                                nc.tensor.matmul(
                                    out=sps[:], lhsT=aext[:],
                                    rhs=bext[0:5, ts(c, 512)],
                                    start=True, stop=True,
                                )
                            nc.scalar.copy(Ssbs[q][:, ts(c, 512)], sps[:])

                    for q in range(group):
                        t = t0 + q
                        p1t, f1T, Ssb = p1ts[q], f1Ts[q], Ssbs[q]

                        # exact fp32 top-8 (we use top-3)
                        m8 = small.tile([128, 8], FP, tag="m8")
                        nc.vector.max(m8[:], Ssb[:])
                        i8 = small.tile([128, 8], mybir.dt.uint32, tag="i8")
                        nc.vector.max_index(i8[:], m8[:], Ssb[:])

                        # weights w = 1/(d+eps), normalized
                        r3 = small.tile([128, 3], FP, tag="r3")
                        nc.scalar.activation(
                            r3[:], m8[:, 0:3], AF.Relu, scale=-1.0
                        )
                        d3 = small.tile([128, 3], FP, tag="d3")
                        nc.scalar.activation(d3[:], r3[:], AF.Sqrt)
                        d3e = small.tile([128, 3], FP, tag="d3e")
                        nc.vector.tensor_scalar_add(d3e[:], d3[:], EPS)
                        w3t = small.tile([128, 3], FP, tag="w3t")
                        nc.vector.reciprocal(w3t[:], d3e[:])
                        ws = small.tile([128, 1], FP, tag="ws")
                        nc.vector.reduce_sum(ws[:], w3t[:], axis=AX.X)
                        rws = small.tile([128, 1], FP, tag="rws")
                        nc.vector.reciprocal(rws[:], ws[:])
                        wn = small.tile([128, 3], FP, tag="wn")
                        nc.vector.tensor_scalar_mul(wn[:], w3t[:], rws[:])

                        # gather the 3 neighbours' feature rows per query
                        g = gpool.tile([128, 3 * C2], FP, tag="g")
                        if batched_gather:
                            nc.gpsimd.indirect_dma_start(
                                out=g[:, :].rearrange(
                                    "p (k c) -> p k c", k=3
                                ),
                                out_offset=None,
                                in_=p2f,
                                in_offset=IndirectOffsetOnAxis(
                                    ap=i8[:, 0:3], axis=0
                                ),
                            )
                        else:
                            for k in range(3):
                                nc.gpsimd.indirect_dma_start(
                                    out=g[:, ts(k, C2)],
                                    out_offset=None,
                                    in_=p2f,
                                    in_offset=IndirectOffsetOnAxis(
                                        ap=i8[:, k : k + 1], axis=0
                                    ),
                                )

                        # interp = sum_k wn_k * g_k
                        veng = nc.gpsimd if stt_gpsimd else nc.vector
                        it0 = ipool.tile([128, C2], FP, tag="it0")
                        nc.scalar.activation(
                            it0[:], g[:, 0:C2], AF.Copy, scale=wn[:, 0:1]
                        )
                        it1 = ipool.tile([128, C2], FP, tag="it1")
                        veng.scalar_tensor_tensor(
                            it1[:], g[:, ts(1, C2)], wn[:, 1:2], it0[:],
                            op0=OP.mult, op1=OP.add,
                        )
                        interp = ipool.tile([128, C2], FP, tag="interp")
                        veng.scalar_tensor_tensor(
                            interp[:], g[:, ts(2, C2)], wn[:, 2:3], it1[:],
                            op0=OP.mult, op1=OP.add,
                        )

                        # channels-on-partitions for the MLP
                        it0p = smpsum.tile([128, 128], FP, tag="sm")
                        nc.tensor.transpose(it0p[:], interp[:, 0:128], ident[:])
                        iT0 = mpool.tile([128, 128], FP, tag="iT0")
                        nc.scalar.copy(iT0[:], it0p[:])
                        it1p = smpsum.tile([128, 128], FP, tag="sm")
                        nc.tensor.transpose(it1p[:], interp[:, 128:256], ident[:])
                        iT1 = mpool.tile([128, 128], FP, tag="iT1")
                        nc.scalar.copy(iT1[:], it1p[:])

                        # layer 1: 320 -> 256 (w1 rows pre-reordered to match
                        # [interp0:128, interp128:256, feats1] chunk order)
                        ps1a = smpsum.tile([128, 128], FP, tag="sm")
                        nc.tensor.matmul(out=ps1a[:], lhsT=mmdt(w1s0[:, 0:128]), rhs=mmdt(iT0[:]), start=True, stop=False)
                        nc.tensor.matmul(out=ps1a[:], lhsT=mmdt(w1s1[:, 0:128]), rhs=mmdt(iT1[:]), start=False, stop=False)
                        nc.tensor.matmul(out=ps1a[:], lhsT=mmdt(w1s2[:, 0:128]), rhs=mmdt(f1T[:]), start=False, stop=True)
                        h1a = mpool.tile([128, 128], FP, tag="h1a")
                        nc.scalar.activation(h1a[:], ps1a[:], AF.Relu, bias=b1a[:])
                        ps1b = smpsum.tile([128, 128], FP, tag="sm")
                        nc.tensor.matmul(out=ps1b[:], lhsT=mmdt(w1s0[:, 128:256]), rhs=mmdt(iT0[:]), start=True, stop=False)
                        nc.tensor.matmul(out=ps1b[:], lhsT=mmdt(w1s1[:, 128:256]), rhs=mmdt(iT1[:]), start=False, stop=False)
                        nc.tensor.matmul(out=ps1b[:], lhsT=mmdt(w1s2[:, 128:256]), rhs=mmdt(f1T[:]), start=False, stop=True)
                        h1b = mpool.tile([128, 128], FP, tag="h1b")
                        nc.scalar.activation(h1b[:], ps1b[:], AF.Relu, bias=b1b[:])

                        # layer 2: 256 -> 128
                        ps2 = smpsum.tile([128, 128], FP, tag="sm")
                        nc.tensor.matmul(out=ps2[:], lhsT=mmdt(w2s0[:]), rhs=mmdt(h1a[:]), start=True, stop=False)
                        nc.tensor.matmul(out=ps2[:], lhsT=mmdt(w2s1[:]), rhs=mmdt(h1b[:]), start=False, stop=True)
                        h2 = mpool.tile([128, 128], FP, tag="h2")
                        nc.scalar.activation(h2[:], ps2[:], AF.Relu, bias=b2s[:])

                        # layer 3: 128 -> 128
                        ps3 = smpsum.tile([128, 128], FP, tag="sm")
                        nc.tensor.matmul(out=ps3[:], lhsT=mmdt(w3s[:]), rhs=mmdt(h2[:]), start=True, stop=True)
                        h3 = mpool.tile([128, 128], FP, tag="h3")
                        nc.scalar.activation(h3[:], ps3[:], AF.Relu, bias=b3s[:])

                        # back to rows-on-partitions, assemble output tile
                        h3p = smpsum.tile([128, 128], FP, tag="sm")
                        nc.tensor.transpose(h3p[:], h3[:], ident[:])
                        outt = opool.tile([128, 3 + H3], FP, tag="outt")
                        nc.scalar.copy(outt[:, 3:], h3p[:])
                        nc.scalar.copy(outt[:, 0:3], p1t[:, 0:3])
                        nc.sync.dma_start(out[ts(t, 128), :], outt[:])

    nc.compile()
    return nc


def make_in_maps(points1, points2, W1, b1, W2, b2, W3, b3, n1s=N1S, ncores=NCORES):
    """Host-side sharding: core c -> (batch c//2, half c%2 of N1)."""
    w1r = np.concatenate([W1[64:192], W1[192:320], W1[0:64]], axis=0).copy()
    shared = {
        "w1": np.ascontiguousarray(w1r, np.float32),
        "w2": np.ascontiguousarray(W2, np.float32),
        "w3": np.ascontiguousarray(W3, np.float32),
        "b1": np.ascontiguousarray(b1.reshape(-1, 1), np.float32),
        "b2": np.ascontiguousarray(b2.reshape(-1, 1), np.float32),
        "b3": np.ascontiguousarray(b3.reshape(-1, 1), np.float32),
    }
    halves = ncores // points1.shape[0]
    in_maps = []
    for c in range(ncores):
        b, h = c // halves, c % halves
        in_maps.append(
            dict(
                shared,
                p1=np.ascontiguousarray(
                    points1[b, h * n1s : (h + 1) * n1s, :], np.float32
                ),
                p2xyzt=np.ascontiguousarray(points2[b, :, :3].T, np.float32),
                p2f=np.ascontiguousarray(points2[b, :, 3:], np.float32),
            )
        )
    return in_maps


_CACHE = {}


def _get_program():
    if "nc" not in _CACHE:
        _CACHE["nc"] = build_program()
    return _CACHE["nc"]


def kernel(points1, points2, W1, b1, W2, b2, W3, b3):
    nc = _get_program()
    in_maps = make_in_maps(points1, points2, W1, b1, W2, b2, W3, b3)
    res = run_bass_kernel_spmd(nc, in_maps, list(range(NCORES)))
    halves = NCORES // B
    out = np.empty((B, N1, 3 + H3), np.float32)
    for c in range(NCORES):
        b, h = c // halves, c % halves
        out[b, h * N1S : (h + 1) * N1S, :] = res.results[c]["out"]
    return out
